# revision 42
# baseline (speedup 1.0000x reference)
"""Trainium2 Bass kernel for nn_DepthWiseConv_AConnect (depthwise 3x3 conv with
per-pool multiplicative weight/bias noise, followed by 8-bit LQuant).

Strategy (8 NeuronCores, data-parallel over the pool axis):
  - Core p handles pool group p: 8 images, Werr[p], Berr[p].
  - Inputs ship as fp16 (products of two 11-bit-mantissa halves are exact in
    the fp32 PSUM accumulator); quantized outputs ship as int8.
  - Host pre-transposes X to channels-major [n, c, h*w] fp16.
  - On device, the depthwise conv is 9 accumulating diagonal matmuls on the
    TensorEngine per PSUM bank tile of 486 output pixels (9 output rows):
    psum[c, pix] += diag(127*w_t)[c] * X^T[c, pix + shift_t].
    Matmuls run tap-outer over 3-bank halves so consecutive instructions
    share the stationary weights.
  - PSUM evict IS the quantization: activation/tensor_scalar compute
    (psum + 127*bias_noisy[c]) with an int8 output dtype.  The hardware
    f32->int8 convert rounds to nearest-even (verified bit-identical to the
    jnp.round magic-constant trick) and saturates to [-128, 127]; the host
    maps the (astronomically rare) -128 back to -127 to match the
    reference's clip-before-scale.  ScalarE drains 2 banks per half and
    VectorE the third, so the post-matmul drain chain is short.
  - Host divides by 127 and transposes back to NHWC.

Performance notes (measured on trn2, core 0):
  - The TensorEngine is the critical path and runs gap-free at full p-state:
    864 matmuls x 486 columns x 0.4167 ns ~= 175 us; LDWEIGHTS pipelines
    behind the column stream.  Columns = out_pixels x 9 taps x C/128 is
    provably minimal for a depthwise conv on the 128-partition PE.  fp8
    DoubleRow was measured to give NO speedup for tap-packing: hardware
    streams the doubled moving rows at 0.5 cyc each (same wall time as one
    fp16 matmul), so a hi/lo-split DR variant ran 301 us despite passing
    the rel-err gate at 6.7e-3.
  - GPSIMD must not touch the data path: a [128, 2916] tensor_scalar costs
    ~43 us there (vs ~1.7 us on VectorE), which made the original version
    DMA-starved and 3.5x slower.
  - The MAGIC bias must be added to fp32 data on-chip; folding it into the
    host-side bias rounds the bias to integers (ulp(1.5*2^23) = 1).
  - Head ~13 us (framework preamble + first loads), tail ~3-5 us (drain +
    epilogue); run-to-run DVFS variance is ~20% (steady matmul 368 ns at
    2.4 GHz vs 441 ns throttled).
  - The warmup chain hands off to the first real matmul at full speed
    (369 ns vs 563-613 ns without it, saving the ~2 us mid-p-state ramp);
    sub-us handoff gaps do NOT reset the hardware p-state even though the
    cost model says they do.  The ramp is DUTY-based: full-width (486-col)
    warmup matmuls ramp the clock in ~3 us where light 64-col ones need
    ~5+ us.
  - Chunk-0's input is split 3 ways and its matmuls run bank-0-first
    (bank 0 needs only input rows 0..10), so the real stream starts at
    ~11.6 us instead of 13.1.
"""
import sys

import numpy as np

try:
    import concourse.bacc as bacc_mod
except ImportError:
    sys.path.insert(0, "/opt/trn_rl_repo")
    import concourse.bacc as bacc_mod

import concourse.mybir as mybir
from concourse.tile import TileContext
from concourse.bass_utils import run_bass_kernel_spmd
from contextlib import ExitStack

POOL = 8
NB = 8            # images per pool group (64 / 8)
H = W = 56
HO = WO = 54
C = 256
NCH = 2           # channel chunks of 128
NPIX = H * W      # 3136
NOUT = HO * WO    # 2916
NBANK = 6         # psum bank tiles per plane (6 * 486 = 2916)
BANKN = 486       # output pixels per psum tile (9 rows x 54)
MAGIC = 12582912.0  # 1.5 * 2^23
S = 127.0

f32 = mybir.dt.float32
f16 = mybir.dt.float16
i8 = mybir.dt.int8
Alu = mybir.AluOpType
Act = mybir.ActivationFunctionType

_cached = {}


def _build():
    nc = bacc_mod.Bacc()
    xt = nc.dram_tensor("xt", [NB, NCH, 128, NPIX], f16, kind="ExternalInput")
    wdg = nc.dram_tensor("wdg", [128, NCH, 9, 128], f16, kind="ExternalInput")
    bv = nc.dram_tensor("bv", [128, NCH, 1], f32, kind="ExternalInput")
    out = nc.dram_tensor("out", [NB, NCH, 128, NOUT], i8, kind="ExternalOutput")

    with TileContext(nc) as tc, ExitStack() as ctx:
        consts = ctx.enter_context(tc.tile_pool(name="consts", bufs=1))
        xpool = ctx.enter_context(tc.tile_pool(name="xpool", bufs=4))
        tpool = ctx.enter_context(tc.tile_pool(name="tpool", bufs=2))
        opool = ctx.enter_context(tc.tile_pool(name="opool", bufs=3))
        pspool = ctx.enter_context(tc.tile_pool(name="pspool", bufs=2, space="PSUM"))
        scrpool = ctx.enter_context(tc.tile_pool(name="scrpool", bufs=1,
                                                 space="PSUM"))

        ws = consts.tile([128, NCH, 9, 128], f16)
        bt = consts.tile([128, NCH, 1], f32)

        # Warm up the TensorEngine p-state during the initial DMA wait: the
        # PE needs ~3us of continuous execution to reach 2.4 GHz, so run a
        # chain of small matmuls on zeroed scratch data spanning past the
        # point where the first image rows land (~13.3us).  Sized long
        # rather than exact: a chain that ends early leaves a PE gap that
        # resets the p-state ramp (measured: an 80-matmul chain ended 1us
        # short and the mid-speed ramp was paid anyway), while overshoot
        # only delays the first real matmul by the overshoot.
        dummy = consts.tile([128, 486], f16)
        nc.vector.memset(dummy, 0.0)
        sps = scrpool.tile([128, 486], f32)
        # Heavy full-width matmuls drive the DVFS ramp; light 64-col ones
        # give the chain a fine-grained end so the handoff gap stays tiny.
        # The governor needs ~4.3us of activity from chain start (~7.3us,
        # the earliest the PE is free) to reach 2.4 GHz, so with the input
        # DMA ready at ~10.5us the handoff is at the 1.2 GHz mid-state and
        # the stream reaches full speed ~2.5us in -- measured equivalent to
        # waiting for a full-speed handoff, and ~1.6us better than no
        # warmup at all.
        for _ in range(7):
            nc.tensor.matmul(sps, lhsT=dummy[:, :128], rhs=dummy,
                             start=True, stop=True, skip_group_check=True)
        for _ in range(3):
            nc.tensor.matmul(sps[:, :64], lhsT=dummy[:, :128],
                             rhs=dummy[:, :64], start=True, stop=True,
                             skip_group_check=True)

        for n in range(NB):
            for q in range(NCH):
                xs = xpool.tile([128, NPIX], f16, tag="xs")
                # split the load so the first half's matmuls (input rows
                # 0..34) don't wait for the whole image; for the very first
                # chunk split finer (bank 0 only needs rows 0..10) so the
                # real matmul stream starts as early as possible
                if n == 0 and q == 0:
                    # DMA-queue packets drain in FIFO order, so sequence the
                    # loads so each matmul dependency lands just-in-time:
                    # rows 0..11 (bank 0's input), taps 0-2, taps 3-8,
                    # rows 12..34, rows 35..55, then everything non-gating
                    nc.sync.dma_start(out=xs[:, :12 * W],
                                      in_=xt[n, q, :, :12 * W])
                    nc.sync.dma_start(out=ws[:, 0, 0:3], in_=wdg[:, 0, 0:3])
                    nc.sync.dma_start(out=ws[:, 0, 3:9], in_=wdg[:, 0, 3:9])
                    nc.sync.dma_start(out=xs[:, 12 * W:35 * W],
                                      in_=xt[n, q, :, 12 * W:35 * W])
                    nc.sync.dma_start(out=xs[:, 35 * W:],
                                      in_=xt[n, q, :, 35 * W:])
                    nc.sync.dma_start(out=ws[:, 1], in_=wdg[:, 1])
                    nc.sync.dma_start(out=bt, in_=bv[:])
                else:
                    nc.sync.dma_start(out=xs[:, :35 * W],
                                      in_=xt[n, q, :, :35 * W])
                    nc.sync.dma_start(out=xs[:, 35 * W:],
                                      in_=xt[n, q, :, 35 * W:])
                xr = xs.rearrange("p (h w) -> p h w", w=W)
                ot = opool.tile([128, NOUT], i8, tag="ot")
                HN = NOUT // 2
                for half in range(2):
                    banks = range(3 * half, 3 * half + 3)
                    pss = [pspool.tile([128, BANKN], f32, tag=f"ps{i}",
                                       name=f"ps{i}")
                           for i in range(3)]
                    if n == NB - 1 and q == NCH - 1 and half == 1:
                        # final half: banks 3,4 first (their drains + DMAs
                        # overlap the rest), then bank 5 split into a 5-row
                        # and a 4-row piece so only a ~216-element
                        # drain->DMA chain remains after the last matmul
                        for bi, b in enumerate((3, 4)):
                            for t in range(9):
                                i, j = divmod(t, 3)
                                rhs = xr[:, 9 * b + i: 9 * b + i + 9,
                                         j: j + 54]
                                nc.tensor.matmul(pss[bi],
                                                 lhsT=ws[:, q, t, :],
                                                 rhs=rhs, start=(t == 0),
                                                 stop=(t == 8),
                                                 skip_group_check=True)
                        ps5b = pspool.tile([128, 4 * 54], f32, tag="ps5b",
                                           name="ps5b", bufs=1)
                        for r0, nr, ps in ((45, 5, pss[2][:, :5 * 54]),
                                           (50, 4, ps5b)):
                            for t in range(9):
                                i, j = divmod(t, 3)
                                rhs = xr[:, r0 + i: r0 + i + nr, j: j + 54]
                                nc.tensor.matmul(ps, lhsT=ws[:, q, t, :],
                                                 rhs=rhs, start=(t == 0),
                                                 stop=(t == 8),
                                                 skip_group_check=True)
                        for bi, b in enumerate((3, 4)):
                            bs = slice(BANKN * b, BANKN * (b + 1))
                            nc.scalar.activation(out=ot[:, bs], in_=pss[bi],
                                                 func=Act.Identity,
                                                 bias=bt[:, q], scale=1.0)
                            nc.sync.dma_start(out=out[n, q, :, bs],
                                              in_=ot[:, bs])
                        for lo, hi, ps in ((45 * 54, 50 * 54,
                                            pss[2][:, :5 * 54]),
                                           (50 * 54, NOUT, ps5b)):
                            nc.vector.tensor_scalar(out=ot[:, lo:hi],
                                                    in0=ps,
                                                    scalar1=bt[:, q],
                                                    scalar2=None,
                                                    op0=Alu.add)
                            nc.sync.dma_start(out=out[n, q, :, lo:hi],
                                              in_=ot[:, lo:hi])
                        continue
                    if n == 0 and q == 0 and half == 0:
                        # bank 0 first: it only needs input rows 0..10, so
                        # its 9 taps run while rows 12..34 are still landing
                        order = [(t, 0) for t in range(9)] + \
                                [(t, b) for t in range(9) for b in (1, 2)]
                    else:
                        order = [(t, b) for t in range(9) for b in banks]
                    for t, b in order:
                        i, j = divmod(t, 3)
                        rhs = xr[:, 9 * b + i: 9 * b + i + 9, j: j + 54]
                        nc.tensor.matmul(pss[b - 3 * half],
                                         lhsT=ws[:, q, t, :],
                                         rhs=rhs, start=(t == 0),
                                         stop=(t == 8),
                                         skip_group_check=True)
                    for bi, b in enumerate(banks):
                        osl = ot[:, BANKN * b: BANKN * (b + 1)]
                        if bi < 2:
                            nc.scalar.activation(out=osl, in_=pss[bi],
                                                 func=Act.Identity,
                                                 bias=bt[:, q], scale=1.0)
                        else:
                            nc.vector.tensor_scalar(out=osl, in0=pss[bi],
                                                    scalar1=bt[:, q],
                                                    scalar2=None, op0=Alu.add)
                    hs = slice(HN * half, HN * (half + 1))
                    nc.sync.dma_start(out=out[n, q, :, hs], in_=ot[:, hs])

    nc.finalize()
    return nc


def kernel(X, W, bias, Werr, Berr, _trace=False):
    X = np.asarray(X, np.float32)
    W = np.asarray(W, np.float32)
    bias = np.asarray(bias, np.float32)
    Werr = np.asarray(Werr, np.float32)
    Berr = np.asarray(Berr, np.float32)

    if "nc" not in _cached:
        _cached["nc"] = _build()
    nc = _cached["nc"]

    Xh = X.astype(np.float16)  # [64, 56, 56, 256]
    w3 = W[..., 0]             # [3, 3, 256]
    we3 = Werr[..., 0]         # [8, 3, 3, 256]

    in_maps = []
    for p in range(POOL):
        xp = Xh[p * NB:(p + 1) * NB].reshape(NB, NPIX, C)
        xp = np.ascontiguousarray(xp.transpose(0, 2, 1)).reshape(NB, NCH, 128, NPIX)

        w_eff = (np.float32(S) * w3 * we3[p]).astype(np.float16)  # [3, 3, 256]
        wdg = np.zeros((NCH, 9, 128, 128), np.float16)
        for q in range(NCH):
            for t in range(9):
                i, j = divmod(t, 3)
                np.fill_diagonal(wdg[q, t], w_eff[i, j, 128 * q:128 * (q + 1)])
        wdg = np.ascontiguousarray(wdg.transpose(2, 0, 1, 3))  # [128,NCH,9,128]

        b_eff = (np.float32(S) * bias * Berr[p]).astype(np.float32)
        bv = np.ascontiguousarray(b_eff.reshape(NCH, 128, 1).transpose(1, 0, 2))
        in_maps.append({"xt": xp, "wdg": wdg, "bv": bv})

    res = run_bass_kernel_spmd(nc, in_maps, core_ids=list(range(POOL)),
                               trace=_trace)
    if _trace:
        _cached["last_result"] = res

    outs = []
    for p in range(POOL):
        o = res.results[p]["out"]  # [NB, NCH, 128, NOUT] int8
        o = np.where(o == -128, np.int8(-127), o).astype(np.float32)
        o = o / np.float32(S)
        o = o.reshape(NB, C, HO, WO).transpose(0, 2, 3, 1)  # NHWC
        outs.append(o)
    return np.ascontiguousarray(np.concatenate(outs, axis=0).astype(np.float32))
